# revision 31
# baseline (speedup 1.0000x reference)
"""Bahdanau additive-attention kernel for Trainium2, 8 NeuronCores.

Problem (B=32, S=2048, H=1024, E=2H):
    hid_proj = hidden @ w_h.T + attn_b                  # (B, H)
    enc_proj[b,s,h] = sum_e enc[b,s,e] * w_e[h,e]       # (B, S, H)   <- dominant
    energy = tanh(hid_proj[:,None,:] + enc_proj)
    scores[b,s] = sum_h energy[b,s,h] * v_w[h]
    attw = softmax(scores, axis=1)
    context[b,e] = sum_s attw[b,s] * enc[b,s,e]

Sharding: data-parallel over batch, 4 batches per core.

Precision plan: the dominant enc_proj matmul is split along H with the
h-dims SORTED BY |v_w| on the host (scores = sum_h energy*v are
invariant to h-permutations).  The 6 low-|v| m-chunks (768 h) run in
fp8 e4m3 with perf_mode=DoubleRow (2 k-chunks per matmul, ~1.7x bf16 PE
throughput); the 2 top-|v| chunks stay bf16.  Because the score error
from chunk m is weighted by its v values, the bottom 6/8 chunks carry
only (6/8)^3 ~ 42% of the error variance — much better than a uniform
split at the same cost.  fp8 operands are host-scaled by powers of two
(TRN e4m3 max normal 240); the ACT tanh input scale divides it out.
The encoder tensor is pre-transposed and pre-cast on the host, so phase
1 uses plain contiguous DMA on the two hardware DGE queues.

Both M=1 matvecs (score v-dot and context) are 4-way column-tiled:
four matmuls run concurrently in disjoint 32-column strips of the PE
array (outputs at PSUM partitions 0/32/64/96, ~4x), and the partial
rows are summed on the DVE via 32-aligned cross-quadrant reads.

Schedule: one flat slot sequence over (batch, s-tile); softmax uses
exp(score) WITHOUT max-subtraction (|scores| ~ 1 by construction), so
exp, the exp-column transposes, and the context matmuls all run
per-s-tile, pipelined: slot q runs phase-1 of slot q, the v-dot of slot
q-1 (so tanh ACTs never stall the PE), and the exp/transpose/context
chainlet of slot q-2 (fully dependence-free by then).  1/Z is folded
into the context drain and the attention-weight output row once Z
finishes.  The kernel tail is just two chainlets instead of a whole
batch's softmax+context.
"""

import numpy as np
import ml_dtypes

import concourse.bass as bass
import concourse.tile as tile
import concourse.mybir as mybir
from concourse import bacc
from concourse.bass_utils import run_bass_kernel_spmd

B, S, H = 32, 2048, 1024
E = 2 * H
NCORES = 8
BL = B // NCORES          # batches per core
P = 128                   # partitions
KC = E // P               # 16 contraction chunks
KH = KC // 2              # k-chunks per half tile
MC = H // P               # 8 h chunks
NT = 512                  # moving free-dim per matmul (1 PSUM bank of fp32)
ST = S // NT              # 4 s-tiles per batch
SJ = S // P               # 16 s-128 blocks per batch
QT = NT // P              # 4 s-128 blocks per s-tile

MF8 = 6                   # low-|v| m-chunks fully in fp8 DoubleRow
MBF = 2                   # top-|v| m-chunks with bf16 (m6 mixed, m7 full)
H8 = (MF8 + 1) * P        # 896: w8 also covers m6's fp8 k-chunks
HB = MBF * P              # 256
KSP = 12                  # m-chunk 6: k-chunks 0..11 fp8 DR, 12..15 bf16

S_E = 32.0                # host scale on enc before e4m3 cast (max ~ 174 < 240)
S_W = 8192.0              # host scale on w_e before e4m3 cast (max ~ 148 < 240)

F32 = mybir.dt.float32
BF16 = mybir.dt.bfloat16
FP8 = mybir.dt.float8e4
BF16_NP = ml_dtypes.bfloat16
FP8_NP = ml_dtypes.float8_e4m3

DR = mybir.MatmulPerfMode.DoubleRow


def build_nc():
    nc = bacc.Bacc("TRN2", target_bir_lowering=False, debug=False)

    enc_in = nc.dram_tensor("encb", [BL, SJ, P, E], BF16, kind="ExternalInput")
    encT8_in = nc.dram_tensor("encT8", [BL, KC, P, S], FP8, kind="ExternalInput")
    encT16_in = nc.dram_tensor("encT16", [BL, KC, P, S], BF16, kind="ExternalInput")
    w8_in = nc.dram_tensor("w8", [KC, P, H8], FP8, kind="ExternalInput")
    w16_in = nc.dram_tensor("w16", [KC, P, HB], BF16, kind="ExternalInput")
    v_in = nc.dram_tensor("vcol", [P, MC], BF16, kind="ExternalInput")
    hid_in = nc.dram_tensor("hidc", [P, BL, MC], F32, kind="ExternalInput")
    ctx_out = nc.dram_tensor("ctx", [BL, E], F32, kind="ExternalOutput")
    attw_out = nc.dram_tensor("attw", [BL, S], F32, kind="ExternalOutput")

    inv_scale = 1.0 / (S_E * S_W)

    with tile.TileContext(nc) as tc:
        with (
            tc.tile_pool(name="singles", bufs=1) as singles,
            tc.tile_pool(name="pT", bufs=2) as pT,
            tc.tile_pool(name="pen", bufs=2) as pen,
            tc.tile_pool(name="pnt", bufs=4) as pnt,
            tc.tile_pool(name="prow", bufs=1) as prow,
            tc.tile_pool(name="pscore", bufs=2) as pscore,
            tc.tile_pool(name="pwcol", bufs=2) as pwcol,
            tc.tile_pool(name="pmm", bufs=2, space="PSUM") as pmm,
            tc.tile_pool(name="psc", bufs=1, space="PSUM") as psc,
            tc.tile_pool(name="pwc", bufs=1, space="PSUM") as pwc,
            tc.tile_pool(name="pctx", bufs=1, space="PSUM") as pctx,
        ):
            # resident weights; only the fp8 weight halves and the bias go
            # ahead of the first slot's encT DMAs — w16/v are issued after
            # them (the bf16 m-chunks run late in the slot)
            w8_sb = singles.tile([P, KC, H8], FP8)
            nc.sync.dma_start(
                out=w8_sb[:, :KH], in_=w8_in[:KH].rearrange("k p h -> p k h")
            )
            nc.sync.dma_start(
                out=w8_sb[:, KH:], in_=w8_in[KH:].rearrange("k p h -> p k h")
            )
            hid_sb = singles.tile([P, BL, MC], F32)
            nc.sync.dma_start(out=hid_sb, in_=hid_in[:, :, :])
            w16_sb = singles.tile([P, KC, HB], BF16)
            nc.sync.dma_start(out=w16_sb, in_=w16_in.rearrange("k p h -> p k h"))
            v_sb = singles.tile([P, MC], BF16)
            nc.sync.dma_start(out=v_sb, in_=v_in[:, :])
            ident = singles.tile([1, 1], F32)
            nc.vector.memset(ident, 1.0)
            # indicator column: 1.0 at partitions 0/32/64/96 — contracts the
            # four column-strip partials in one K=128 matmul
            ind4 = singles.tile([P, 1], BF16)
            nc.vector.memset(ind4, 0.0)
            for q in range(4):
                nc.vector.memset(ind4[32 * q : 32 * q + 1], 1.0)

            en_t = {}     # (b, st) -> energy tile
            srow = {}     # b -> scores row tile
            ex_t = {}     # b -> exp row tile
            zt_t = {}     # b -> per-s-tile Z partials
            rz_t = {}     # b -> 1/Z
            wcol_t = {}   # b -> exp column tile
            pw_t = {}     # b -> exp transpose PSUM tile
            cps_t = {}    # b -> context partial PSUM tile (4 banks)
            nt_t = {}     # (b, st) -> natural-layout enc tile

            def nt_prefetch(b, st):
                nt = pnt.tile([P, QT, E], BF16, name=f"nt_{b}_{st}", tag="nt")
                nc.sync.dma_start(
                    out=nt,
                    in_=enc_in[b, st * QT : (st + 1) * QT].rearrange("q p e -> p q e"),
                )
                nt_t[(b, st)] = nt

            def phase1(b, st):
                # encT tiles split in k-halves so matmuls can start after
                # half a tile has landed; spread across both HWDGE queues
                sl = slice(st * NT, (st + 1) * NT)
                e8 = [
                    pT.tile([P, KH, NT], FP8, name=f"e8{h}_{b}_{st}", tag=f"e8{h}")
                    for h in range(2)
                ]
                e16 = [
                    pT.tile([P, KH, NT], BF16, name=f"e16{h}_{b}_{st}", tag=f"e16{h}")
                    for h in range(2)
                ]
                nc.scalar.dma_start(
                    out=e8[0], in_=encT8_in[b, :KH, :, sl].rearrange("k p s -> p k s")
                )
                nc.scalar.dma_start(
                    out=e8[1], in_=encT8_in[b, KH:, :, sl].rearrange("k p s -> p k s")
                )
                nc.scalar.dma_start(
                    out=e16[0],
                    in_=encT16_in[b, :KH, :, sl].rearrange("k p s -> p k s"),
                )
                nc.sync.dma_start(
                    out=e16[1],
                    in_=encT16_in[b, KH:, :, sl].rearrange("k p s -> p k s"),
                )
                en = pen.tile([P, MC, NT], BF16, name=f"en_{b}_{st}", tag="en")
                for m in range(MC):
                    ps = pmm.tile([P, NT], F32, name=f"ps_{b}_{st}_{m}", tag="ps")
                    ndr = (KC if m < MF8 else KSP if m == MF8 else 0) // 2
                    for k in range(ndr):
                        h, kk = divmod(2 * k, KH)
                        nc.tensor.matmul(
                            ps,
                            lhsT=w8_sb[:, 2 * k : 2 * k + 2, m * P : (m + 1) * P],
                            rhs=e8[h][:, kk : kk + 2, :],
                            start=(k == 0),
                            stop=(k == ndr - 1 and m != MF8),
                            perf_mode=DR,
                        )
                    mm = m - MF8
                    for k in range(2 * ndr if m == MF8 else 0 if m < MF8 else 0, KC):
                        if m < MF8:
                            break
                        h, kk = divmod(k, KH)
                        nc.tensor.matmul(
                            ps,
                            lhsT=w16_sb[:, k, mm * P : (mm + 1) * P],
                            rhs=e16[h][:, kk, :],
                            start=(k == 0 and ndr == 0),
                            stop=(k == KC - 1),
                        )
                    nc.scalar.activation(
                        out=en[:, m, :],
                        in_=ps,
                        func=mybir.ActivationFunctionType.Tanh,
                        bias=hid_sb[:, b, m : m + 1],
                        scale=inv_scale,
                    )
                en_t[(b, st)] = en

            def vdot(b, st):
                # 4-way column-tiled: m -> strip m%4, partial at partition
                # 32*(m%4); two accumulating matmuls per strip, then a DVE
                # cross-quadrant reduce into the scores row.
                en = en_t.pop((b, st))
                sc = psc.tile([P, NT], F32, name=f"sc_{b}_{st}", tag="sc")
                for m in range(MC):
                    q = m % 4
                    nc.tensor.matmul(
                        sc[32 * q : 32 * q + 1, :],
                        lhsT=v_sb[:, m : m + 1],
                        rhs=en[:, m, :],
                        start=(m < 4),
                        stop=(m >= 4),
                        tile_position=(0, 32 * q),
                        skip_group_check=True,
                    )
                acc = prow.tile([1, NT], F32, name=f"sacc_{b}_{st}", tag="sacc")
                nc.vector.tensor_copy(out=acc, in_=sc[0:1, :])
                nc.vector.tensor_add(acc, acc, sc[32:33, :])
                nc.vector.tensor_add(acc, acc, sc[64:65, :])
                nc.vector.tensor_add(
                    srow[b][:, st * NT : (st + 1) * NT], acc, sc[96:97, :]
                )

            def chainlet(b, st):
                # per-s-tile softmax tail: exp chunk (+Z partial), 4 PE
                # transposes into exp columns, 16 col-tiled context matmuls
                if st == 0:
                    ex_t[b] = prow.tile([1, S], F32, name=f"ex_{b}", tag="ex")
                    zt_t[b] = prow.tile([1, ST], F32, name=f"zt_{b}", tag="zt")
                    pw_t[b] = pwc.tile([P, SJ], F32, name=f"pw_{b}", tag="pw")
                    wcol_t[b] = pwcol.tile([P, SJ], BF16, name=f"wc_{b}", tag="wc")
                    cps_t[b] = pctx.tile([P, 4, NT], F32, name=f"cps_{b}", tag="cps")
                    if b == BL - 1:
                        # the PE-side strip reduce in combine() contracts all
                        # 128 partitions; zero the 124 the matmuls never
                        # write (PSUM garbage can be NaN, and 0*NaN = NaN)
                        nc.vector.memset(cps_t[b], 0.0)
                ex, zt = ex_t[b], zt_t[b]
                pw, wcols, cps = pw_t[b], wcol_t[b], cps_t[b]
                sl = slice(st * NT, (st + 1) * NT)
                nc.scalar.activation(
                    out=ex[:, sl],
                    in_=srow[b][:, sl],
                    func=mybir.ActivationFunctionType.Exp,
                    accum_out=zt[:, st : st + 1],
                )
                for q in range(QT):
                    j = st * QT + q
                    nc.tensor.transpose(
                        pw[:, j : j + 1], ex[:, j * P : (j + 1) * P], ident
                    )
                nc.vector.tensor_copy(
                    out=wcols[:, st * QT : (st + 1) * QT],
                    in_=pw[:, st * QT : (st + 1) * QT],
                )
                nt = nt_t.pop((b, st))
                for g in range(4):
                    for q in range(QT):
                        j = st * QT + q
                        nc.tensor.matmul(
                            cps[32 * q : 32 * q + 1, g, :],
                            lhsT=wcols[:, j : j + 1],
                            rhs=nt[:, q, g * NT : (g + 1) * NT],
                            start=(st == 0),
                            stop=(st == ST - 1),
                            tile_position=(0, 32 * q),
                            skip_group_check=True,
                        )

            def combine(b):
                # Z, 1/Z, the normalized attention-weight output row, and
                # the context cross-strip reduce (wide multi-bank DVE ops)
                zt = zt_t.pop(b)
                z = prow.tile([1, 1], F32, name=f"z_{b}", tag="z")
                nc.vector.tensor_add(z, zt[:, 0:1], zt[:, 1:2])
                nc.vector.tensor_add(z, z, zt[:, 2:3])
                nc.vector.tensor_add(z, z, zt[:, 3:4])
                rz = prow.tile([1, 1], F32, name=f"rz_{b}", tag="rz")
                nc.vector.reciprocal(out=rz, in_=z)

                # attw row on the DVE — an ACT Identity here would delay the
                # next slot's tanh drains on the scalar queue
                wrow = prow.tile([1, S], F32, name=f"wr_{b}", tag="wr")
                nc.vector.tensor_scalar_mul(wrow, ex_t.pop(b), rz)
                nc.sync.dma_start(out=attw_out[b], in_=wrow)

                cps = cps_t.pop(b)
                wcol_t.pop(b)
                pw_t.pop(b)
                ctxrow = prow.tile([1, E], F32, name=f"cr_{b}", tag="cr")
                if b == BL - 1:
                    # last batch: the serial DVE chain would be exposed at
                    # the kernel tail — contract the strips on the PE via
                    # the indicator column instead
                    csb = prow.tile([P, 4, NT], BF16, name=f"csb_{b}", tag="csb")
                    nc.vector.tensor_copy(out=csb, in_=cps)
                    red = psc.tile([P, NT], F32, name=f"red_{b}", tag="sc")
                    for g in range(4):
                        # concurrent column strips — one bank, disjoint
                        # partitions, so the 4 reduces don't serialize on
                        # the DVE drains
                        nc.tensor.matmul(
                            red[32 * g : 32 * g + 1, :],
                            lhsT=ind4,
                            rhs=csb[:, g, :],
                            tile_position=(0, 32 * g),
                            skip_group_check=True,
                        )
                    for g in range(4):
                        nc.vector.tensor_scalar_mul(
                            ctxrow[:, g * NT : (g + 1) * NT],
                            red[32 * g : 32 * g + 1, :],
                            rz,
                        )
                else:
                    acc = prow.tile([1, E], F32, name=f"ctacc_{b}", tag="ctacc")
                    nc.vector.tensor_copy(out=acc, in_=cps[0:1, :, :])
                    nc.vector.tensor_add(acc, acc, cps[32:33, :, :])
                    nc.vector.tensor_add(acc, acc, cps[64:65, :, :])
                    nc.vector.tensor_add(acc, acc, cps[96:97, :, :])
                    nc.vector.tensor_scalar_mul(ctxrow, acc, rz)
                nc.sync.dma_start(out=ctx_out[b], in_=ctxrow)

            # flat slot pipeline: slot i runs phase1(i), vdot(i-1),
            # chainlet(i-2) [+ combine when a batch's last chainlet ran]
            slots = [(b, st) for b in range(BL) for st in range(ST)]

            def run_lagged(i):
                if 1 <= i <= len(slots):
                    vdot(*slots[i - 1])
                if i >= 2:
                    bq, sq = slots[i - 2]
                    chainlet(bq, sq)
                    if sq == ST - 1:
                        combine(bq)

            for i, (b, st) in enumerate(slots):
                if st == 0:
                    srow[b] = pscore.tile([1, S], F32, name=f"sr_{b}", tag="sr")
                phase1(b, st)
                nt_prefetch(b, st)
                run_lagged(i)

            n = len(slots)
            run_lagged(n)
            run_lagged(n + 1)

    nc.compile()
    return nc


_CACHE = {}


def _get_nc():
    if "nc" not in _CACHE:
        _CACHE["nc"] = build_nc()
    return _CACHE["nc"]


def prep_in_maps(hidden, encoder_outputs, attn_w, attn_b, v_w):
    hidden = np.asarray(hidden, dtype=np.float32)
    enc = np.asarray(encoder_outputs, dtype=np.float32)
    attn_w = np.asarray(attn_w, dtype=np.float32)
    attn_b = np.asarray(attn_b, dtype=np.float32)
    v_w = np.asarray(v_w, dtype=np.float32)

    # host-side prep of the small operands; h-dims sorted by |v_w| so the
    # 6 low-|v| m-chunks (where fp8 error is v-damped) run in fp8
    perm = np.argsort(np.abs(v_w))
    w_h = attn_w[:, :H]                       # (H, H)
    w_e = attn_w[perm, H:]                    # (H, E), h-permuted
    hid_proj = (hidden @ w_h.T + attn_b)[:, perm]  # (B, H) fp32, exact
    v_p = v_w[perm]

    wTs = np.ascontiguousarray(w_e.T * S_W)   # (E, H), scaled
    w8 = np.ascontiguousarray(wTs[:, :H8]).astype(FP8_NP).reshape(KC, P, H8)
    w16 = (
        np.ascontiguousarray(wTs[:, MF8 * P :]).astype(BF16_NP).reshape(KC, P, HB)
    )
    vcol = np.ascontiguousarray(v_p.reshape(MC, P).T).astype(BF16_NP)  # (P, MC)

    # encoder tensor: bf16 natural layout (context matmul) and scaled
    # transposed layouts (phase-1) in fp8 and bf16
    encb = enc.astype(BF16_NP).reshape(B, SJ, P, E)
    encTs = np.ascontiguousarray((enc * S_E).transpose(0, 2, 1))  # (B, E, S)
    encT8 = encTs.astype(FP8_NP).reshape(B, KC, P, S)
    encT16 = encTs.astype(BF16_NP).reshape(B, KC, P, S)

    in_maps = []
    for c in range(NCORES):
        hp = hid_proj[c * BL : (c + 1) * BL]  # (BL, H)
        # hidc[p, b, m] = hid_proj[b, m*128+p]
        hidc = np.ascontiguousarray(hp.reshape(BL, MC, P).transpose(2, 0, 1))
        in_maps.append(
            {
                "encb": encb[c * BL : (c + 1) * BL],
                "encT8": encT8[c * BL : (c + 1) * BL],
                "encT16": encT16[c * BL : (c + 1) * BL],
                "w8": w8,
                "w16": w16,
                "vcol": vcol,
                "hidc": hidc.astype(np.float32),
            }
        )
    return in_maps


def kernel(hidden, encoder_outputs, attn_w, attn_b, v_w):
    in_maps = prep_in_maps(hidden, encoder_outputs, attn_w, attn_b, v_w)
    nc = _get_nc()
    res = run_bass_kernel_spmd(nc, in_maps, core_ids=list(range(NCORES)))
    ctx = np.concatenate([res.results[c]["ctx"] for c in range(NCORES)], axis=0)
    attw = np.concatenate([res.results[c]["attw"] for c in range(NCORES)], axis=0)
    return ctx.astype(np.float32), attw.astype(np.float32)
